# revision 19
# baseline (speedup 1.0000x reference)
"""RWKV ChannelMixer (single-token) on 8 Trainium2 NeuronCores.

Math (reference):
    xn  = LayerNorm(x) * ln_w + ln_b
    xk  = xn*tmk + prev*(1-tmk);  xr = xn*tmr + prev*(1-tmr)
    r   = sigmoid(rw @ xr)                       # (D,)
    k   = relu(kw @ xk)^2                        # (F,)
    out = x + r * (vw @ k)                       # (D,)
    returns (out, xn)

Sharding (8 cores, no collectives):
    kw: F-row-sharded (512 rows/core), vw: F-col-sharded (512 cols/core),
    rw: D-row-sharded (128 rows/core).  Host computes the O(D) LN/token-
    mix prologue and the final unshard: v = sum_i v_i/(SV*SK^2),
    r = sigmoid(concat(r_i)/SR), out = x + r*v.

Device kernel (per core) -- memory-bound: weights ship as fp8-e3m4
(1.2MB/core, rel err ~9e-3) and every matvec runs with the WEIGHTS as
the stationary matmul operand: 128x128 fp8 tiles stream through the PE
fast-weight-load path (~26ns/tile dispatch) while the one-column fp16
activation vector is the moving operand.

Critical-path facts this layout is built around (measured):
  - a DMA completion semaphore lags its last byte by ~1.4us while the
    weight stream still occupies HBM, ~0.6us once it has drained; so
    kw streams FIRST (its semaphore gates the longest dependent chain
    k -> v) and vw is split so the two lags roughly balance,
  - the PE executes its queue in order, so instructions are emitted in
    exact DMA-arrival order and nothing data-independent sits between
    a chunk semaphore and its matmuls,
  - [128 x small] DMAs decay to per-partition descriptors, so outputs
    are PE-transposed to row-major and leave as ONE [9,128] fp16 DMA
    (v in rows 0-7, pre-sigmoid r in row 8),
  - the k epilogue is a single DVE op: k*max(k,0) == relu(k)^2 with
    the fp8 scale folded into the host unshard.
"""

import sys
import numpy as np
import ml_dtypes

for _p in ("/opt/trn_rl_repo", "/root/.axon_site/_ro/trn_rl_repo"):
    if _p not in sys.path:
        sys.path.append(_p)

D = 1024
F = 4096
N_CORES = 8
FSH = F // N_CORES      # 512 kw rows / vw cols per core
DSH = D // N_CORES      # 128 rw rows per core
LN_EPS = 1e-5

# weight quantization: fp8-e3m4 with per-tensor scales (keeps values in
# the normal range: max|w*s| < 15.5 for all three)
WDT = "e3m4"            # "e3m4" | "f16"  (f16 = accuracy fallback)
SK, SV, SR = 48.0, 128.0, 64.0

# the big weights SBUF tile, in 128-column tile units (consumption order):
#   [0:32)   kwT   tile (fb, c)  = kw_shard[fb*128:(fb+1)*128, c*128:..].T
#   [32:40)  rwT   tile c        = rw_shard[:, c*128:(c+1)*128].T
#   [40:72)  vwT   tile (dc, fb) = vw[dc*128:(dc+1)*128, f0+fb*128:..].T
N_TILES = 72
# DMA chunks in tile units; kw split so its matmuls start early, vw
# split so the in-stream semaphore lag (dc0-3) and the end-of-stream
# lag (dc4-7) balance
WCH = [(0, 16), (16, 40), (40, 56), (56, 72)]

_STATE = {}


def _body(nc, tc, mybir, stage):
    f32 = mybir.dt.float32
    f16 = mybir.dt.float16
    wdt = mybir.dt.float8e3 if WDT == "e3m4" else f16
    Alu = mybir.AluOpType
    Act = mybir.ActivationFunctionType

    wt_ds = [nc.dram_tensor(f"wt_p{j}", [128, (b - a) * 128], wdt,
                            kind="ExternalInput").ap()
             for j, (a, b) in enumerate(WCH)]
    # smalls, row-major: rows 0-7 = xk d-chunks, rows 8-15 = xr d-chunks
    sm_d = nc.dram_tensor("smalls", [16, 128], f32, kind="ExternalInput").ap()
    # combined output, row-major fp16: rows 0-7 = v*SV, row 8 = r*SR
    vr_d = nc.dram_tensor("vr_out", [9, 128], f16, kind="ExternalOutput").ap()

    import contextlib
    with contextlib.ExitStack() as ctx:
        wp = ctx.enter_context(tc.tile_pool(name="w", bufs=1))
        vp = ctx.enter_context(tc.tile_pool(name="v", bufs=1))
        pp = ctx.enter_context(tc.tile_pool(name="ps", bufs=1, space="PSUM"))

        # ---- DMA: smalls via the GpSimd SWDGE queue (its own issue path
        # and DMA queue -- the scalar HWDGE ring showed ~1.7us first-byte
        # latency, and putting it on the SP ring delays every weight
        # chunk by its issue time); weight chunks on the SP ring in
        # consumption order
        sm_sb = vp.tile([16, 128], f32, tag="sm")
        nc.gpsimd.dma_start(out=sm_sb[:], in_=sm_d[:])
        wt_sb = wp.tile([128, N_TILES * 128], wdt, tag="wt")
        if stage >= 2:
            for j, (a, b) in enumerate(WCH):
                nc.sync.dma_start(out=wt_sb[:, a * 128:b * 128], in_=wt_ds[j][:])

        # ---- constants; dummy activation forces the ACT table load early
        ones_c = vp.tile([1, 1], f32, tag="ones_c")
        nc.vector.memset(ones_c[:], 1.0)
        dummy = vp.tile([1, 1], f32, tag="dummy")
        nc.scalar.activation(dummy[:], ones_c[0:1, 0:1], Act.Relu)
        from concourse.masks import make_identity
        ident = vp.tile([16, 16], f32, tag="ident")
        make_identity(nc, ident)
        ident_h = vp.tile([128, 128], f16, tag="identh")
        make_identity(nc, ident_h)

        # ---- PSUM tiles
        sm_ps = pp.tile([128, 16], f32, tag="smT", bufs=1)
        r_ps = pp.tile([128, 1], f32, tag="rps", bufs=1)
        k_ps = pp.tile([128, 4], f32, tag="kps", bufs=1)
        v_ps = pp.tile([128, 8], f32, tag="vps", bufs=1)
        vrT_ps = pp.tile([9, 128], f16, tag="vrT", bufs=1)

        # ---- transpose smalls into moving-operand layout [d128, 16]
        nc.tensor.transpose(sm_ps[:], sm_sb[:], ident[:])
        xkT = vp.tile([128, 16], f16, tag="xkT")
        nc.vector.tensor_copy(xkT[:], sm_ps[:])   # cols 0-7 xk, 8-15 xr

        if stage < 2:
            return

        def wtile(u):
            return wt_sb[:, u * 128:(u + 1) * 128]

        # ---- k_raw[fb] = (kw*SK) @ xk per f-block, then ONE DVE op for
        # the whole epilogue: kT_h = k_raw * max(k_raw, 0) = SK^2 * k
        kT_h = vp.tile([128, 4], f16, tag="kTh")
        k_sb = vp.tile([128, 4], f32, tag="ksb")
        for fb in range(4):
            for c in range(8):
                nc.tensor.matmul(k_ps[:, fb:fb + 1], wtile(fb * 8 + c),
                                 xkT[:, c:c + 1],
                                 start=(c == 0), stop=(c == 7))
        # both ops on the DVE queue: no cross-engine semaphore hop
        nc.vector.tensor_copy(k_sb[:], k_ps[:])
        nc.vector.scalar_tensor_tensor(out=kT_h[:], in0=k_sb[:], scalar=0.0,
                                       in1=k_sb[:], op0=Alu.max, op1=Alu.mult)

        # ---- r_raw = (rw*SR) @ xr : 8 accumulating stationary tiles;
        # r joins the v transpose at the end (row 8), so no extra DMA
        for c in range(8):
            nc.tensor.matmul(r_ps[:], wtile(32 + c), xkT[:, 8 + c:9 + c],
                             start=(c == 0), stop=(c == 7))

        if stage < 3:
            return

        # ---- v_raw = (vw*SV) @ kT_h : one PSUM column per d-chunk; the
        # 8 accumulation groups are SEQUENTIAL (dc outer) because a
        # group's start=True marks the whole 2KB psum zero region
        # pending-zero (interleaved groups corrupt each other).
        for dc in range(8):
            for fb in range(4):
                nc.tensor.matmul(v_ps[:, dc:dc + 1], wtile(40 + dc * 4 + fb),
                                 kT_h[:, fb:fb + 1],
                                 start=(fb == 0), stop=(fb == 3))

        # ---- tail: gather v (scaled back by 1/SK^2) + r into [128, 9]
        # fp16, one PE transpose, one [9,128] DMA
        vr_sb = vp.tile([128, 9], f16, tag="vr")
        nc.vector.tensor_scalar_mul(vr_sb[:, 0:8], v_ps[:], 1.0 / (SK * SK))
        nc.scalar.copy(vr_sb[:, 8:9], r_ps[:])
        nc.tensor.transpose(vrT_ps[:], vr_sb[:], ident_h[:])
        outvr = vp.tile([9, 128], f16, tag="ovr")
        nc.vector.tensor_copy(outvr[:], vrT_ps[:])
        nc.sync.dma_start(out=vr_d[:], in_=outvr[:])


def _build(stage=3):
    import concourse.bacc as bacc
    import concourse.tile as tile
    from concourse import mybir

    nc = bacc.Bacc("TRN2", target_bir_lowering=False, debug=False,
                   num_devices=N_CORES)
    with tile.TileContext(nc) as tc:
        _body(nc, tc, mybir, stage)
    nc.compile()
    return nc


def _quant(a):
    if WDT == "e3m4":
        return np.clip(a, -15.5, 15.5).astype(ml_dtypes.float8_e3m4)
    return a.astype(np.float16)


def _prep_shared(kw, vw, rw):
    """Pack per-core weights as transposed 128x128 stationary tiles.

    Returns per-core dicts of dram-tensor name -> array (the big tile is
    split into per-chunk tensors so each DMA reads contiguous DRAM).
    """
    maps = []
    for i in range(N_CORES):
        tiles = np.empty((128, N_TILES * 128), dtype=np.float32)
        kw_s = kw[i * FSH:(i + 1) * FSH, :]                  # [512, 1024]
        for fb in range(4):
            for c in range(8):
                u = fb * 8 + c
                tiles[:, u * 128:(u + 1) * 128] = (
                    kw_s[fb * 128:(fb + 1) * 128, c * 128:(c + 1) * 128].T * SK)
        rw_s = rw[i * DSH:(i + 1) * DSH, :]                  # [128, 1024]
        for c in range(8):
            u = 32 + c
            tiles[:, u * 128:(u + 1) * 128] = rw_s[:, c * 128:(c + 1) * 128].T * SR
        vw_s = vw[:, i * FSH:(i + 1) * FSH]                  # [1024, 512]
        for dc in range(8):
            for fb in range(4):
                u = 40 + dc * 4 + fb
                tiles[:, u * 128:(u + 1) * 128] = (
                    vw_s[dc * 128:(dc + 1) * 128, fb * 128:(fb + 1) * 128].T * SV)
        q = _quant(tiles)
        m = {}
        for j, (a, b) in enumerate(WCH):
            m[f"wt_p{j}"] = np.ascontiguousarray(q[:, a * 128:b * 128])
        maps.append(m)
    return maps


def _prep_smalls(x, state, tmk, tmr, lnw, lnb):
    """Host LN + token mix; returns [16, 128] fp32 (xk rows | xr rows)."""
    mu = float(x.mean())
    var = float(np.square(x - mu).mean())
    xn = (x - mu) / np.sqrt(var + LN_EPS) * lnw + lnb
    prev = state[0]
    xk = xn * tmk + prev * (1.0 - tmk)
    xr = xn * tmr + prev * (1.0 - tmr)
    sm = np.concatenate([xk.reshape(8, 128), xr.reshape(8, 128)], axis=0)
    return np.ascontiguousarray(sm).astype(np.float32)


def kernel(x, state, time_mix_k, time_mix_r, kw, vw, rw, ln_weight, ln_bias):
    from concourse import bass_utils

    x = np.asarray(x, dtype=np.float32)
    state = np.asarray(state, dtype=np.float32)
    kw = np.asarray(kw, dtype=np.float32)
    vw = np.asarray(vw, dtype=np.float32)
    rw = np.asarray(rw, dtype=np.float32)
    tmk = np.asarray(time_mix_k, dtype=np.float32)
    tmr = np.asarray(time_mix_r, dtype=np.float32)
    lnw = np.asarray(ln_weight, dtype=np.float32)
    lnb = np.asarray(ln_bias, dtype=np.float32)

    if "nc" not in _STATE:
        _STATE["nc"] = _build()
    nc = _STATE["nc"]

    maps = _prep_shared(kw, vw, rw)
    sm = _prep_smalls(x, state, tmk, tmr, lnw, lnb)
    in_maps = [dict(maps[i], smalls=sm) for i in range(N_CORES)]

    res = bass_utils.run_bass_kernel_spmd(nc, in_maps, core_ids=list(range(N_CORES)))

    # unshard: v = sum of partials / SV, r = sigmoid(concat / SR)
    v = np.zeros(D, dtype=np.float64)
    r_pre = np.empty(D, dtype=np.float64)
    for i in range(N_CORES):
        arr = res.results[i]["vr_out"].astype(np.float64)
        v += arr[0:8].reshape(D) / SV
        r_pre[i * DSH:(i + 1) * DSH] = arr[8] / SR
    r = 1.0 / (1.0 + np.exp(-r_pre))
    out = x + (r * v).astype(np.float32)

    # xn: exact fp32 LN on host (auxiliary state output)
    mu = float(x.mean())
    var = float(np.square(x - mu).mean())
    xn = (x - mu) / np.sqrt(var + LN_EPS) * lnw + lnb
    return np.asarray(out, dtype=np.float32), np.asarray(xn, dtype=np.float32)
